# revision 20
# baseline (speedup 1.0000x reference)
"""Trainium2 Bass kernel for the ExomaAttention (DCT-kernelized attention) module.

Full-input contract: kernel(**inputs) takes the unsharded inputs and returns
the full [32, 128, 4096] float32 output.

Sharding: pure data-parallel over batch. 8 cores x 4 batches each. Each core
runs an identical Bass program; only the activation shard (hidden_states^T)
differs per core. Weights are replicated. No collectives.

Math notes (validated against the reference in numpy):
  * kv_write_indices == arange(128) == S, so the kv caches are fully
    overwritten by the projected k/v; the k_cache/v_cache/mask inputs are dead.
  * q-side DCT projection folds into the weights on the host:
      qp = (X @ Wq_h) @ proj = X @ (Wq_h @ proj)  per head block h,
    so the Q projection directly produces softmax-ready qp in [token, e]
    layout. The k-side cannot fold (proj contracts the token axis there).
  * Per (b, kv-head g):   kp = k^T @ proj; keyp = softmax_rows(kp)
                          scoresT = v^T @ keyp^T        (one PE transpose of keyp)
    Per (b, head h), g=h//4: query = softmax_rows(qp)
                          out2T[j,i] = sum_t query[t,j] * scoresT[t,i]
    attnT[h*128+j, b*128+i] = out2T[j,i];  out = attnT^T @ o_w
  * All matmul operands are fp16 (fp32 PSUM accumulation): 4x faster PE than
    fp32, ~9e-4 end-to-end relative error.

Perf notes (v1):
  * Weights and xT are repacked HOST-SIDE into chunk-contiguous layout so
    each weight DMA moves a [128, 8*512] quarter-chunk with 8 KiB contiguous
    per-partition lines (vs 1 KiB lines when slicing the row-major matrix).
    The v0 kernel's DMA engines were ~98% time-active at ~1 KiB/packet
    (packet-overhead-bound), causing startup PE stalls and HAM re-throttles.
  * kv-chunk PSUM drains alternate ACT/DVE engines and scoresT emission is
    split per v-half so it trails kv chunks 2/3 immediately (kills the
    ~2us PSUM-wait stall + ~10us half-rate HAM window at the q transition).
  * The last o-proj chunk runs b-outer (contraction completes per batch
    tile 4x earlier) so the final PSUM drains + output DMAs overlap the
    remaining batches' matmuls: tail shrinks from ~7.4us to ~3.5us.
"""

import numpy as np

import concourse.bass as bass
import concourse.mybir as mybir
import concourse.tile as tile
from concourse import bacc
from concourse.bass_utils import run_bass_kernel_spmd
from concourse.masks import make_identity

FP16 = mybir.dt.float16
F32 = mybir.dt.float32
AX_X = mybir.AxisListType.X
EXP = mybir.ActivationFunctionType.Exp

N_CORES = 8
B, T, H = 32, 128, 4096
NH, NKV, HD = 32, 8, 128
B_LOC = B // N_CORES          # 4 batches per core
TOK = B_LOC * T               # 512 tokens per core
KT = H // 128                 # 32 contraction tiles
QC = NH * HD                  # 4096 q columns
KVC = 2 * NKV * HD            # 2048 k+v columns
NQ = 4                        # quarters per 512-col chunk (8 k-tiles each)
KQ = KT // NQ                 # k-tiles per quarter


def _build_program():
    nc = bacc.Bacc("TRN2", target_bir_lowering=False, debug=False)
    # xT packed host-side as [p, k, tok] so one DMA piece has 8 KiB lines.
    xT_d = nc.dram_tensor("xT", [128, KT * TOK], FP16, kind="ExternalInput").ap()
    # weights packed host-side as [chunk, p, kt, 512] (chunk-contiguous).
    wqp_d = nc.dram_tensor("wqp", [QC // 512, 128, KT * 512], FP16,
                           kind="ExternalInput").ap()
    wkv_d = nc.dram_tensor("wkv", [KVC // 512, 128, KT * 512], FP16,
                           kind="ExternalInput").ap()
    wo_d = nc.dram_tensor("wo", [H // 512, 128, KT * 512], FP16,
                          kind="ExternalInput").ap()
    proj_d = nc.dram_tensor("proj", [HD, HD], FP16, kind="ExternalInput").ap()
    out_d = nc.dram_tensor("out", [TOK, H], F32, kind="ExternalOutput").ap()

    with tile.TileContext(nc) as tc:
        _emit(tc, nc, xT_d, wqp_d, wkv_d, wo_d, proj_d, out_d)
    nc.compile()
    return nc


def _emit(tc, nc, xT_d, wqp_d, wkv_d, wo_d, proj_d, out_d):
    from contextlib import ExitStack

    ctx = ExitStack()
    with ctx:
        persist = ctx.enter_context(tc.tile_pool(name="persist", bufs=1))
        wstream = ctx.enter_context(tc.tile_pool(name="wstream", bufs=6))
        small = ctx.enter_context(tc.tile_pool(name="small", bufs=8))
        psum = ctx.enter_context(tc.tile_pool(name="psum", bufs=8, space="PSUM"))

        # ---- resident tiles -------------------------------------------------
        xT_sb = persist.tile([128, KT * TOK], FP16, name="xT_sb", tag="xT_sb")
        xT_v = xT_sb.rearrange("p (k n) -> p k n", n=TOK)          # [128, 32, 512]
        xT_src = xT_d.rearrange("p (k n) -> p k n", n=TOK)
        proj_sb = persist.tile([128, HD], FP16, name="proj_sb", tag="proj_sb")
        # Startup DMA choreography. kv chunk 0 consumes (xT slab k, wkv0
        # tile k) in k order at ~300 GB/s — close to the aggregate DMA
        # ceiling — so both streams are cut into 2-slab pairs (2 KiB lines)
        # and fed in consumption order on two parallel queues: xT on sync
        # (whose HW ring starts ~2.7us earlier), wkv0 on scalar. The first
        # two wkv0 pairs ride on sync so the scalar ring's late start
        # doesn't gate k=0.
        nc.sync.dma_start(out=proj_sb[:], in_=proj_d[:])
        wkv0_q = [wstream.tile([128, KQ * 512], FP16, name=f"wkv_0_{q}", tag="w")
                  for q in range(NQ)]

        def _wkv0_dma(eng, k0, k1):
            q, c0 = k0 // KQ, (k0 % KQ) * 512
            eng.dma_start(out=wkv0_q[q][:, c0:c0 + (k1 - k0) * 512],
                          in_=wkv_d[0, :, k0 * 512:k1 * 512])

        # sync (ring live ~2.7us before scalar's): single-slab DMAs for k=0,1
        # of both streams so the very first matmul unblocks ~2us earlier,
        # then the rest of xT; scalar: the rest of wkv chunk 0. 2-slab pairs
        # (2 KiB lines) up to k=8, 4-slab quads beyond.
        for k in (0, 1):
            nc.sync.dma_start(out=xT_v[:, k:k + 1, :], in_=xT_src[:, k:k + 1, :])
            _wkv0_dma(nc.sync, k, k + 1)
        for k0, k1 in ((2, 4), (4, 6), (6, 8), (8, 12), (12, 16), (16, 20),
                       (20, 24), (24, 28), (28, 32)):
            nc.sync.dma_start(out=xT_v[:, k0:k1, :], in_=xT_src[:, k0:k1, :])
        for k0, k1 in ((2, 4), (4, 6), (6, 8), (8, 12), (12, 16), (16, 20),
                       (20, 24), (24, 28), (28, 32)):
            _wkv0_dma(nc.scalar, k0, k1)

        ident = persist.tile([128, 128], FP16, name="ident", tag="ident")
        make_identity(nc, ident[:])

        # PE warm-up: dummy matmuls with no DMA dependency keep the PE busy
        # (and the HAM clock-gate warming) while the first input DMAs land.
        warm = persist.tile([128, 512], FP16, name="warm", tag="warm")
        nc.vector.memset(warm[:], 0.0)
        warm_ps = psum.tile([128, 512], F32, name="warm_ps", tag="ps")
        for _ in range(8):
            nc.tensor.matmul(warm_ps[:], ident[:], warm[:], start=True, stop=True)

        KV_sb = persist.tile([128, B_LOC * KVC], FP16, name="KV_sb", tag="KV_sb")
        attnT_sb = persist.tile([128, NH * TOK], FP16, name="attnT_sb", tag="attnT_sb")
        # scoresT per (b, g): [128, 128] at column (b*NKV+g)*128
        sT_sb = persist.tile([128, B_LOC * NKV * 128], FP16, name="sT_sb", tag="sT_sb")

        def kv_slice(b, col, width=128):
            return KV_sb[:, b * KVC + col: b * KVC + col + width]

        # ---- weight streaming ----------------------------------------------
        def dma_quarters(w_d, ci, pfx, pre_all=None):
            """Fetch chunk ci of a packed weight tensor as 4 quarter tiles."""
            bufs = []
            for q in range(NQ):
                if pre_all is not None:
                    bufs.append(pre_all[q].rearrange("p (k n) -> p k n", n=512))
                    continue
                wt = wstream.tile([128, KQ * 512], FP16,
                                  name=f"{pfx}_{ci}_{q}", tag="w")
                nc.sync.dma_start(out=wt[:],
                                  in_=w_d[ci, :, q * KQ * 512:(q + 1) * KQ * 512])
                bufs.append(wt.rearrange("p (k n) -> p k n", n=512))
            return bufs

        def drain_halves(dst, ps_tile, b):
            """PSUM->SBUF drain split across ACT+DVE so per-batch latency is
            halved and downstream PE consumers unblock sooner."""
            if b % 2 == 0:
                nc.scalar.copy(dst[:, :256], ps_tile[:, :256])
                nc.vector.tensor_copy(dst[:, 256:], ps_tile[:, 256:])
            else:
                nc.vector.tensor_copy(dst[:, :256], ps_tile[:, :256])
                nc.scalar.copy(dst[:, 256:], ps_tile[:, 256:])

        # ---- stage emitters -------------------------------------------------
        def emit_kv_chunk(ci, pre_all=None, tail_cb=None, post_cb=None):
            """KV[:, ci*512:(ci+1)*512] = X @ Wkv chunk for all local batches.

            tail_cb: emitted after k-tile KT-3 — PE work there (e.g. keyp
            transposes) overlaps the chunk's last k-tiles so its cross-engine
            copies complete before the chunk's PSUMs drain.
            post_cb(b): emitted right after batch b's drain, so per-batch
            consumers (kp / scoresT matmuls) start as soon as THEIR columns
            exist instead of after all four drains."""
            wq = dma_quarters(wkv_d, ci, "wkv", pre_all=pre_all)
            ps = [psum.tile([128, 512], F32, name=f"kvps_{ci}_{b}", tag="ps")
                  for b in range(B_LOC)]
            for k in range(KT):
                wt = wq[k // KQ][:, k % KQ, :]
                for b in range(B_LOC):
                    nc.tensor.matmul(ps[b][:],
                                     xT_v[:, k, b * 128:(b + 1) * 128],
                                     wt,
                                     start=(k == 0), stop=(k == KT - 1))
                if tail_cb is not None and k == KT - 6:
                    tail_cb()
            # All drains first: the engine queues are FIFO, so any post work
            # (kp/sT chains with heavy ACT/DVE ops) emitted between drains
            # would delay the later batches' drains and stall the PE.
            for b in range(B_LOC):
                drain_halves(kv_slice(b, ci * 512, 512), ps[b], b)
            if post_cb is not None:
                for b in range(B_LOC):
                    post_cb(b)

        def softmax_quad(ps_tile, pfx, copy_first=True, alt=0,
                         out_tag="soft", out_bufs=48):
            """Row-softmax of 4 [128,128] slices of a [128,512] PSUM tile.
            copy_first: one DVE copy frees the PSUM bank early; the chain then
            runs off the SBUF copy (use where PSUM slot reuse gates the PE)."""
            if copy_first:
                sb = small.tile([128, 512], F32, name=f"{pfx}_sb", tag="smsb",
                                bufs=8)
                nc.vector.tensor_copy(sb[:], ps_tile[:])
                ps_tile = sb
            negmax = small.tile([128, 4], F32, name=f"{pfx}_nm", tag="negmax")
            nc.vector.reduce_max(negmax[:],
                                 ps_tile.rearrange("p (h e) -> p h e", e=128),
                                 axis=AX_X, negate=True)
            exb = small.tile([128, 512], F32, name=f"{pfx}_exb", tag="exp",
                             bufs=8)
            exps = []
            for i in range(4):
                ex = exb[:, i * 128:(i + 1) * 128]
                nc.scalar.activation(ex, ps_tile[:, i * 128:(i + 1) * 128],
                                     EXP, bias=negmax[:, i:i + 1])
                exps.append(ex)
            sums = small.tile([128, 4], F32, name=f"{pfx}_sum", tag="sums")
            nc.vector.reduce_sum(sums[:],
                                 exb.rearrange("p (h e) -> p h e", e=128),
                                 axis=AX_X)
            recip = small.tile([128, 4], F32, name=f"{pfx}_rcp", tag="recip")
            nc.vector.reciprocal(recip[:], sums[:])
            outs = []
            for i in range(4):
                sm = small.tile([128, 128], FP16, name=f"{pfx}_sm{i}",
                                tag=out_tag, bufs=out_bufs)
                if (i + alt) % 2 == 0:
                    nc.vector.tensor_scalar_mul(sm[:], exps[i][:],
                                                recip[:, i:i + 1])
                else:
                    nc.scalar.mul(sm[:], exps[i][:], recip[:, i:i + 1])
                outs.append(sm)
            return outs

        def emit_q_chunk(grp):
            """qp for heads 4*grp..4*grp+3, all batches, + softmax -> query tiles.

            qp[t, e] = X @ Wq' directly (proj folded into Wq on the host), in
            [token, e] layout, which is exactly the out2T lhsT layout.
            """
            queries = {}
            wq = dma_quarters(wqp_d, grp, "wqp")
            ps = [psum.tile([128, 512], F32, name=f"qps_{grp}_{b}", tag="ps")
                  for b in range(B_LOC)]
            for k in range(KT):
                wt = wq[k // KQ][:, k % KQ, :]
                for b in range(B_LOC):
                    nc.tensor.matmul(ps[b][:],
                                     xT_v[:, k, b * 128:(b + 1) * 128],
                                     wt,
                                     start=(k == 0), stop=(k == KT - 1))
            for b in range(B_LOC):
                sms = softmax_quad(ps[b], f"qp{grp}{b}", alt=b)
                for hi in range(4):
                    queries[(b, hi)] = sms[hi]
            return queries

        def emit_kp_half(b, half):
            """kp + softmax -> keyp tiles for kv heads 4*half..4*half+3.
            half 0 reads only KV chunk 0, half 1 only chunk 1, so each can be
            emitted as soon as its chunk exists."""
            kp_ps = psum.tile([128, 512], F32, name=f"kp_{b}_{half}", tag="ps")
            for gi in range(4):
                g = half * 4 + gi
                nc.tensor.matmul(kp_ps[:, gi * 128:(gi + 1) * 128],
                                 kv_slice(b, g * 128),
                                 proj_sb[:],
                                 start=True, stop=True)
            # copy_first=False: skips a 690ns DVE copy per (b, half) in the
            # post-chunk window where DVE queue latency gates the PE.
            return softmax_quad(kp_ps, f"kp{b}{half}", copy_first=False,
                                alt=half, out_tag="keyp", out_bufs=32)

        def emit_sT_pre(b, half, keyp_half):
            """keyp transposes + PSUM->SBUF copies for (b, half). Emitted in
            the tail of the kv chunk producing this half's v heads, so the
            copies complete while the chunk's last k-tiles still run."""
            kT_ps = psum.tile([128, 512], FP16, name=f"kT_{b}_{half}", tag="ps")
            for gi in range(4):
                nc.tensor.transpose(kT_ps[:, gi * 128:(gi + 1) * 128],
                                    keyp_half[gi][:], ident[:])
            kTs = []
            for gi in range(4):
                kt = small.tile([128, 128], FP16, name=f"kT_{b}_{half}_{gi}",
                                tag="kTs", bufs=16)
                if (gi + b) % 2 == 0:
                    nc.scalar.copy(kt[:], kT_ps[:, gi * 128:(gi + 1) * 128])
                else:
                    nc.vector.tensor_copy(kt[:],
                                          kT_ps[:, gi * 128:(gi + 1) * 128])
                kTs.append(kt)
            return kTs

        def emit_sT_post(b, half, kTs):
            """scoresT matmuls for (b, half); needs the kv chunk's drained
            xv columns (lhsT) + the pre-copied kT tiles (rhs)."""
            sT_ps = psum.tile([128, 512], F32, name=f"sT_{b}_{half}", tag="ps")
            for gi in range(4):
                g = half * 4 + gi
                nc.tensor.matmul(sT_ps[:, gi * 128:(gi + 1) * 128],
                                 kv_slice(b, NKV * HD + g * 128),
                                 kTs[gi][:],
                                 start=True, stop=True)
            for gi in range(4):
                g = half * 4 + gi
                col = (b * NKV + g) * 128
                if (gi + b) % 2 == 0:
                    nc.vector.tensor_copy(sT_sb[:, col:col + 128],
                                          sT_ps[:, gi * 128:(gi + 1) * 128])
                else:
                    nc.scalar.copy(sT_sb[:, col:col + 128],
                                   sT_ps[:, gi * 128:(gi + 1) * 128])

        def emit_out2T(grp, queries):
            """out2T for the 4 heads of group grp (kv head g == grp)."""
            g = grp
            for b in range(B_LOC):
                o2_ps = psum.tile([128, 512], F32, name=f"o2_{grp}_{b}", tag="ps")
                scol = (b * NKV + g) * 128
                for hi in range(4):
                    nc.tensor.matmul(o2_ps[:, hi * 128:(hi + 1) * 128],
                                     queries[(b, hi)][:],
                                     sT_sb[:, scol:scol + 128],
                                     start=True, stop=True)
                for hi in range(4):
                    h = grp * 4 + hi
                    dst = attnT_sb[:, h * TOK + b * 128: h * TOK + (b + 1) * 128]
                    if (hi + b) % 2 == 0:
                        nc.vector.tensor_copy(dst, o2_ps[:, hi * 128:(hi + 1) * 128])
                    else:
                        nc.scalar.copy(dst, o2_ps[:, hi * 128:(hi + 1) * 128])

        def emit_oproj_chunk(co, engine_alt, mid_cbs=()):
            """out[:, co*512:(co+1)*512] = attnT^T @ Wo chunk.

            mid_cbs: [(split_at, cb), ...] — cb emitted after a-tiles below
            split_at; the preceding matmuls are PE filler while the late
            attention chains finish."""
            wq = dma_quarters(wo_d, co, "wo")
            ps = [psum.tile([128, 512], F32, name=f"ops_{co}_{b}", tag="ps")
                  for b in range(B_LOC)]
            bounds = [0] + [s for s, _ in mid_cbs] + [KT]
            cbs = [cb for _, cb in mid_cbs] + [None]
            for rng, cb in zip([range(a, b) for a, b in zip(bounds, bounds[1:])],
                               cbs):
                for a in rng:
                    wt = wq[a // KQ][:, a % KQ, :]
                    for b in range(B_LOC):
                        nc.tensor.matmul(ps[b][:],
                                         attnT_sb[:, a * TOK + b * 128: a * TOK + (b + 1) * 128],
                                         wt,
                                         start=(a == 0), stop=(a == KT - 1))
                if cb is not None:
                    cb()
            for b in range(B_LOC):
                ost = small.tile([128, 512], F32, name=f"ost_{co}_{b}", tag="ost",
                                 bufs=6)
                if (b + engine_alt) % 2 == 0:
                    nc.scalar.copy(ost[:], ps[b][:])
                else:
                    nc.vector.tensor_copy(ost[:], ps[b][:])
                nc.gpsimd.dma_start(out=out_d[b * 128:(b + 1) * 128,
                                              co * 512:(co + 1) * 512],
                                    in_=ost[:])

        def emit_oproj_last(co):
            """Final o-proj chunk, b-outer: each batch tile's contraction
            finishes 1/4-chunk apart so its drain + output DMA overlap the
            remaining batches' matmuls. The very last batch fans its drain +
            output DMA across engines/queues in 128-col slices to minimize
            the post-matmul tail."""
            wq = dma_quarters(wo_d, co, "wo")
            for b in range(B_LOC - 1):
                ps = psum.tile([128, 512], F32, name=f"ops_{co}_{b}", tag="ps")
                for a in range(KT):
                    nc.tensor.matmul(ps[:],
                                     attnT_sb[:, a * TOK + b * 128: a * TOK + (b + 1) * 128],
                                     wq[a // KQ][:, a % KQ, :],
                                     start=(a == 0), stop=(a == KT - 1))
                ost = small.tile([128, 512], F32, name=f"ost_{co}_{b}", tag="ost",
                                 bufs=6)
                # Spread the earlier batches' 1MB outputs across three queues
                # so no single queue has a serialized multi-MB flush right
                # before the kernel-end barrier.
                if b % 2 == 0:
                    nc.scalar.copy(ost[:], ps[:])
                else:
                    nc.vector.tensor_copy(ost[:], ps[:])
                [nc.gpsimd, nc.scalar, nc.sync][b].dma_start(
                    out=out_d[b * 128:(b + 1) * 128,
                              co * 512:(co + 1) * 512],
                    in_=ost[:])
            # Final batch: four 128-column contraction passes, so the first
            # 3/4 of its output drains + DMAs while the PE still runs the
            # remaining passes; only the last 0.25MB is exposed in the tail.
            b = B_LOC - 1
            ps = psum.tile([128, 512], F32, name=f"ops_{co}_{b}", tag="ps")
            for ci4 in range(4):
                cs = slice(ci4 * 128, (ci4 + 1) * 128)
                for a in range(KT):
                    nc.tensor.matmul(ps[:, cs],
                                     attnT_sb[:, a * TOK + b * 128: a * TOK + (b + 1) * 128],
                                     wq[a // KQ][:, a % KQ, ci4 * 128:(ci4 + 1) * 128],
                                     start=(a == 0), stop=(a == KT - 1))
                ostq = small.tile([128, 128], F32, name=f"ostq_{co}_{ci4}",
                                  tag="ostq", bufs=2)
                if ci4 % 2 == 0:
                    nc.scalar.copy(ostq[:], ps[:, cs])
                else:
                    nc.vector.tensor_copy(ostq[:], ps[:, cs])
                [nc.sync, nc.scalar][ci4 % 2].dma_start(
                    out=out_d[b * 128:(b + 1) * 128,
                              co * 512 + ci4 * 128: co * 512 + (ci4 + 1) * 128],
                    in_=ostq[:])

        # ---- schedule -------------------------------------------------------
        # Attention stages trail their producers by >= one ~28us PE stage so
        # the cross-engine softmax chains stay off the PE critical path.
        kp_half0, kp_half1 = {}, {}
        emit_kv_chunk(0, pre_all=wkv0_q,
                      post_cb=lambda b: kp_half0.__setitem__(
                          b, emit_kp_half(b, 0)))
        emit_kv_chunk(1,
                      post_cb=lambda b: kp_half1.__setitem__(
                          b, emit_kp_half(b, 1)))

        kTs0, kTs1 = {}, {}

        def _sT0_pre():
            for b in range(B_LOC):
                kTs0[b] = emit_sT_pre(b, 0, kp_half0[b])

        def _sT1_pre():
            for b in range(B_LOC):
                kTs1[b] = emit_sT_pre(b, 1, kp_half1[b])

        emit_kv_chunk(2, tail_cb=_sT0_pre,
                      post_cb=lambda b: emit_sT_post(b, 0, kTs0[b]))
        emit_kv_chunk(3, tail_cb=_sT1_pre,
                      post_cb=lambda b: emit_sT_post(b, 1, kTs1[b]))
        queries = {0: emit_q_chunk(0)}
        queries[1] = emit_q_chunk(1)
        emit_out2T(0, queries.pop(0))
        queries[2] = emit_q_chunk(2)
        emit_out2T(1, queries.pop(1))
        queries[3] = emit_q_chunk(3)
        emit_out2T(2, queries.pop(2))
        queries[4] = emit_q_chunk(4)
        emit_out2T(3, queries.pop(3))
        queries[5] = emit_q_chunk(5)
        emit_out2T(4, queries.pop(4))
        queries[6] = emit_q_chunk(6)
        emit_out2T(5, queries.pop(5))
        queries[7] = emit_q_chunk(7)
        emit_out2T(6, queries.pop(6))

        def _tail_g7():
            emit_out2T(7, queries.pop(7))

        emit_oproj_chunk(0, 0, mid_cbs=[(16, _tail_g7)])
        for co in range(1, H // 512 - 1):
            emit_oproj_chunk(co, co % 2)
        emit_oproj_last(H // 512 - 1)


_NC_CACHE = None


def _get_program():
    global _NC_CACHE
    if _NC_CACHE is None:
        _NC_CACHE = _build_program()
    return _NC_CACHE


def _pack_chunks(w):
    """[H, C] row-major -> [C/512, 128, KT*512] chunk-contiguous fp16."""
    C = w.shape[1]
    return np.ascontiguousarray(
        w.reshape(KT, 128, C // 512, 512).transpose(2, 1, 0, 3)
        .reshape(C // 512, 128, KT * 512)).astype(np.float16)


def kernel(hidden_states, k_cache=None, v_cache=None, mask=None, qkv_w=None,
           o_w=None, proj=None, kv_write_indices=None, **_ignored):
    hidden_states = np.asarray(hidden_states, dtype=np.float32)
    qkv_w = np.asarray(qkv_w, dtype=np.float32)
    o_w = np.asarray(o_w, dtype=np.float32)
    proj = np.asarray(proj, dtype=np.float32)

    # Fold the DCT projection into the q-side weights (exact in fp32).
    wq = qkv_w[:, :QC]
    wqp = (wq.reshape(H, NH, HD) @ proj).reshape(H, QC)
    wqp16 = _pack_chunks(wqp)
    wkv16 = _pack_chunks(qkv_w[:, QC:])
    wo16 = _pack_chunks(o_w)
    proj16 = proj.astype(np.float16)

    in_maps = []
    for c in range(N_CORES):
        shard = hidden_states[c * B_LOC:(c + 1) * B_LOC]          # [4,128,4096]
        # pack xT as [p, k, tok]: row h = k*128+p, tok = b*128+t
        xT = np.ascontiguousarray(
            shard.reshape(B_LOC, T, KT, 128).transpose(3, 2, 0, 1)
            .reshape(128, KT * TOK))
        in_maps.append({
            "xT": xT.astype(np.float16),
            "wqp": wqp16,
            "wkv": wkv16,
            "wo": wo16,
            "proj": proj16,
        })

    nc = _get_program()
    res = run_bass_kernel_spmd(nc, in_maps, list(range(N_CORES)))

    out = np.empty((B, T, H), np.float32)
    for c in range(N_CORES):
        out[c * B_LOC:(c + 1) * B_LOC] = res.results[c]["out"].reshape(B_LOC, T, H)
    kernel.last_results = res
    return out


# revision 22
# speedup vs baseline: 1.0030x; 1.0030x over previous
"""Trainium2 Bass kernel for the ExomaAttention (DCT-kernelized attention) module.

Full-input contract: kernel(**inputs) takes the unsharded inputs and returns
the full [32, 128, 4096] float32 output.

Sharding: pure data-parallel over batch. 8 cores x 4 batches each. Each core
runs an identical Bass program; only the activation shard (hidden_states^T)
differs per core. Weights are replicated. No collectives.

Math notes (validated against the reference in numpy):
  * kv_write_indices == arange(128) == S, so the kv caches are fully
    overwritten by the projected k/v; the k_cache/v_cache/mask inputs are dead.
  * q-side DCT projection folds into the weights on the host:
      qp = (X @ Wq_h) @ proj = X @ (Wq_h @ proj)  per head block h,
    so the Q projection directly produces softmax-ready qp in [token, e]
    layout. The k-side cannot fold (proj contracts the token axis there).
  * Per (b, kv-head g):   kp = k^T @ proj; keyp = softmax_rows(kp)
                          scoresT = v^T @ keyp^T        (one PE transpose of keyp)
    Per (b, head h), g=h//4: query = softmax_rows(qp)
                          out2T[j,i] = sum_t query[t,j] * scoresT[t,i]
    attnT[h*128+j, b*128+i] = out2T[j,i];  out = attnT^T @ o_w
  * All matmul operands are fp16 (fp32 PSUM accumulation): 4x faster PE than
    fp32, ~9e-4 end-to-end relative error.

Perf notes (v1):
  * Weights and xT are repacked HOST-SIDE into chunk-contiguous layout so
    each weight DMA moves a [128, 8*512] quarter-chunk with 8 KiB contiguous
    per-partition lines (vs 1 KiB lines when slicing the row-major matrix).
    The v0 kernel's DMA engines were ~98% time-active at ~1 KiB/packet
    (packet-overhead-bound), causing startup PE stalls and HAM re-throttles.
  * kv-chunk PSUM drains alternate ACT/DVE engines and scoresT emission is
    split per v-half so it trails kv chunks 2/3 immediately (kills the
    ~2us PSUM-wait stall + ~10us half-rate HAM window at the q transition).
  * The last o-proj chunk runs b-outer (contraction completes per batch
    tile 4x earlier) so the final PSUM drains + output DMAs overlap the
    remaining batches' matmuls: tail shrinks from ~7.4us to ~3.5us.
"""

import numpy as np

import concourse.bass as bass
import concourse.mybir as mybir
import concourse.tile as tile
from concourse import bacc
from concourse.bass_utils import run_bass_kernel_spmd
from concourse.masks import make_identity

FP16 = mybir.dt.float16
F32 = mybir.dt.float32
AX_X = mybir.AxisListType.X
EXP = mybir.ActivationFunctionType.Exp

N_CORES = 8
B, T, H = 32, 128, 4096
NH, NKV, HD = 32, 8, 128
B_LOC = B // N_CORES          # 4 batches per core
TOK = B_LOC * T               # 512 tokens per core
KT = H // 128                 # 32 contraction tiles
QC = NH * HD                  # 4096 q columns
KVC = 2 * NKV * HD            # 2048 k+v columns
NQ = 4                        # quarters per 512-col chunk (8 k-tiles each)
KQ = KT // NQ                 # k-tiles per quarter


def _build_program():
    nc = bacc.Bacc("TRN2", target_bir_lowering=False, debug=False)
    # xT packed host-side as [p, k, tok] so one DMA piece has 8 KiB lines.
    xT_d = nc.dram_tensor("xT", [128, KT * TOK], FP16, kind="ExternalInput").ap()
    # weights packed host-side as [chunk, p, kt, 512] (chunk-contiguous).
    wqp_d = nc.dram_tensor("wqp", [QC // 512, 128, KT * 512], FP16,
                           kind="ExternalInput").ap()
    wkv_d = nc.dram_tensor("wkv", [KVC // 512, 128, KT * 512], FP16,
                           kind="ExternalInput").ap()
    wo_d = nc.dram_tensor("wo", [H // 512, 128, KT * 512], FP16,
                          kind="ExternalInput").ap()
    proj_d = nc.dram_tensor("proj", [HD, HD], FP16, kind="ExternalInput").ap()
    out_d = nc.dram_tensor("out", [TOK, H], F32, kind="ExternalOutput").ap()

    with tile.TileContext(nc) as tc:
        _emit(tc, nc, xT_d, wqp_d, wkv_d, wo_d, proj_d, out_d)
    nc.compile()
    return nc


def _emit(tc, nc, xT_d, wqp_d, wkv_d, wo_d, proj_d, out_d):
    from contextlib import ExitStack

    ctx = ExitStack()
    with ctx:
        persist = ctx.enter_context(tc.tile_pool(name="persist", bufs=1))
        wstream = ctx.enter_context(tc.tile_pool(name="wstream", bufs=6))
        small = ctx.enter_context(tc.tile_pool(name="small", bufs=8))
        psum = ctx.enter_context(tc.tile_pool(name="psum", bufs=8, space="PSUM"))

        # ---- resident tiles -------------------------------------------------
        xT_sb = persist.tile([128, KT * TOK], FP16, name="xT_sb", tag="xT_sb")
        xT_v = xT_sb.rearrange("p (k n) -> p k n", n=TOK)          # [128, 32, 512]
        xT_src = xT_d.rearrange("p (k n) -> p k n", n=TOK)
        proj_sb = persist.tile([128, HD], FP16, name="proj_sb", tag="proj_sb")
        # Startup DMA choreography. kv chunk 0 consumes (xT slab k, wkv0
        # tile k) in k order at ~300 GB/s — close to the aggregate DMA
        # ceiling — so both streams are cut into 2-slab pairs (2 KiB lines)
        # and fed in consumption order on two parallel queues: xT on sync
        # (whose HW ring starts ~2.7us earlier), wkv0 on scalar. The first
        # two wkv0 pairs ride on sync so the scalar ring's late start
        # doesn't gate k=0.
        nc.sync.dma_start(out=proj_sb[:], in_=proj_d[:])
        wkv0_q = [wstream.tile([128, KQ * 512], FP16, name=f"wkv_0_{q}", tag="w")
                  for q in range(NQ)]

        def _wkv0_dma(eng, k0, k1):
            q, c0 = k0 // KQ, (k0 % KQ) * 512
            eng.dma_start(out=wkv0_q[q][:, c0:c0 + (k1 - k0) * 512],
                          in_=wkv_d[0, :, k0 * 512:k1 * 512])

        # sync (ring live ~2.7us before scalar's): single-slab DMAs for k=0,1
        # of both streams so the very first matmul unblocks ~2us earlier,
        # then the rest of xT; scalar: the rest of wkv chunk 0. 2-slab pairs
        # (2 KiB lines) up to k=8, 4-slab quads beyond.
        for k in (0, 1):
            nc.sync.dma_start(out=xT_v[:, k:k + 1, :], in_=xT_src[:, k:k + 1, :])
            _wkv0_dma(nc.sync, k, k + 1)
        for k0, k1 in ((2, 4), (4, 6), (6, 8), (8, 12), (12, 16), (16, 20),
                       (20, 24), (24, 28), (28, 32)):
            nc.sync.dma_start(out=xT_v[:, k0:k1, :], in_=xT_src[:, k0:k1, :])
        for k0, k1 in ((2, 4), (4, 6), (6, 8), (8, 12), (12, 16), (16, 20),
                       (20, 24), (24, 28), (28, 32)):
            _wkv0_dma(nc.scalar, k0, k1)

        ident = persist.tile([128, 128], FP16, name="ident", tag="ident")
        make_identity(nc, ident[:])

        # PE warm-up: dummy matmuls with no DMA dependency keep the PE busy
        # (and the HAM clock-gate warming) while the first input DMAs land.
        warm = persist.tile([128, 512], FP16, name="warm", tag="warm")
        nc.vector.memset(warm[:], 0.0)
        warm_ps = psum.tile([128, 512], F32, name="warm_ps", tag="ps")
        for _ in range(8):
            nc.tensor.matmul(warm_ps[:], ident[:], warm[:], start=True, stop=True)

        KV_sb = persist.tile([128, B_LOC * KVC], FP16, name="KV_sb", tag="KV_sb")
        attnT_sb = persist.tile([128, NH * TOK], FP16, name="attnT_sb", tag="attnT_sb")
        # scoresT per (b, g): [128, 128] at column (b*NKV+g)*128
        sT_sb = persist.tile([128, B_LOC * NKV * 128], FP16, name="sT_sb", tag="sT_sb")

        def kv_slice(b, col, width=128):
            return KV_sb[:, b * KVC + col: b * KVC + col + width]

        # ---- weight streaming ----------------------------------------------
        def dma_quarters(w_d, ci, pfx, pre_all=None):
            """Fetch chunk ci of a packed weight tensor as 4 quarter tiles."""
            bufs = []
            for q in range(NQ):
                if pre_all is not None:
                    bufs.append(pre_all[q].rearrange("p (k n) -> p k n", n=512))
                    continue
                wt = wstream.tile([128, KQ * 512], FP16,
                                  name=f"{pfx}_{ci}_{q}", tag="w")
                nc.sync.dma_start(out=wt[:],
                                  in_=w_d[ci, :, q * KQ * 512:(q + 1) * KQ * 512])
                bufs.append(wt.rearrange("p (k n) -> p k n", n=512))
            return bufs

        def drain_halves(dst, ps_tile, b):
            """PSUM->SBUF drain split across ACT+DVE so per-batch latency is
            halved and downstream PE consumers unblock sooner."""
            if b % 2 == 0:
                nc.scalar.copy(dst[:, :256], ps_tile[:, :256])
                nc.vector.tensor_copy(dst[:, 256:], ps_tile[:, 256:])
            else:
                nc.vector.tensor_copy(dst[:, :256], ps_tile[:, :256])
                nc.scalar.copy(dst[:, 256:], ps_tile[:, 256:])

        # ---- stage emitters -------------------------------------------------
        def emit_kv_chunk(ci, pre_all=None, tail_cb=None, post_cb=None):
            """KV[:, ci*512:(ci+1)*512] = X @ Wkv chunk for all local batches.

            tail_cb: emitted after k-tile KT-3 — PE work there (e.g. keyp
            transposes) overlaps the chunk's last k-tiles so its cross-engine
            copies complete before the chunk's PSUMs drain.
            post_cb(b): emitted right after batch b's drain, so per-batch
            consumers (kp / scoresT matmuls) start as soon as THEIR columns
            exist instead of after all four drains."""
            wq = dma_quarters(wkv_d, ci, "wkv", pre_all=pre_all)
            ps = [psum.tile([128, 512], F32, name=f"kvps_{ci}_{b}", tag="ps")
                  for b in range(B_LOC)]
            for k in range(KT):
                wt = wq[k // KQ][:, k % KQ, :]
                for b in range(B_LOC):
                    nc.tensor.matmul(ps[b][:],
                                     xT_v[:, k, b * 128:(b + 1) * 128],
                                     wt,
                                     start=(k == 0), stop=(k == KT - 1))
                if tail_cb is not None and k == KT - 6:
                    tail_cb()
            # All drains first: the engine queues are FIFO, so any post work
            # (kp/sT chains with heavy ACT/DVE ops) emitted between drains
            # would delay the later batches' drains and stall the PE.
            for b in range(B_LOC):
                drain_halves(kv_slice(b, ci * 512, 512), ps[b], b)
            if post_cb is not None:
                for b in range(B_LOC):
                    post_cb(b)

        def softmax_quad(ps_tile, pfx, copy_first=True, alt=0,
                         out_tag="soft", out_bufs=48):
            """Row-softmax of 4 [128,128] slices of a [128,512] PSUM tile.
            copy_first: one DVE copy frees the PSUM bank early; the chain then
            runs off the SBUF copy (use where PSUM slot reuse gates the PE)."""
            if copy_first:
                sb = small.tile([128, 512], F32, name=f"{pfx}_sb", tag="smsb",
                                bufs=8)
                nc.vector.tensor_copy(sb[:], ps_tile[:])
                ps_tile = sb
            negmax = small.tile([128, 4], F32, name=f"{pfx}_nm", tag="negmax")
            nc.vector.reduce_max(negmax[:],
                                 ps_tile.rearrange("p (h e) -> p h e", e=128),
                                 axis=AX_X, negate=True)
            exb = small.tile([128, 512], F32, name=f"{pfx}_exb", tag="exp",
                             bufs=8)
            exps = []
            for i in range(4):
                ex = exb[:, i * 128:(i + 1) * 128]
                nc.scalar.activation(ex, ps_tile[:, i * 128:(i + 1) * 128],
                                     EXP, bias=negmax[:, i:i + 1])
                exps.append(ex)
            sums = small.tile([128, 4], F32, name=f"{pfx}_sum", tag="sums")
            nc.vector.reduce_sum(sums[:],
                                 exb.rearrange("p (h e) -> p h e", e=128),
                                 axis=AX_X)
            recip = small.tile([128, 4], F32, name=f"{pfx}_rcp", tag="recip")
            nc.vector.reciprocal(recip[:], sums[:])
            outs = []
            for i in range(4):
                sm = small.tile([128, 128], FP16, name=f"{pfx}_sm{i}",
                                tag=out_tag, bufs=out_bufs)
                if (i + alt) % 2 == 0:
                    nc.vector.tensor_scalar_mul(sm[:], exps[i][:],
                                                recip[:, i:i + 1])
                else:
                    nc.scalar.mul(sm[:], exps[i][:], recip[:, i:i + 1])
                outs.append(sm)
            return outs

        def emit_q_chunk(grp):
            """qp for heads 4*grp..4*grp+3, all batches, + softmax -> query tiles.

            qp[t, e] = X @ Wq' directly (proj folded into Wq on the host), in
            [token, e] layout, which is exactly the out2T lhsT layout.
            """
            queries = {}
            wq = dma_quarters(wqp_d, grp, "wqp")
            ps = [psum.tile([128, 512], F32, name=f"qps_{grp}_{b}", tag="ps")
                  for b in range(B_LOC)]
            for k in range(KT):
                wt = wq[k // KQ][:, k % KQ, :]
                for b in range(B_LOC):
                    nc.tensor.matmul(ps[b][:],
                                     xT_v[:, k, b * 128:(b + 1) * 128],
                                     wt,
                                     start=(k == 0), stop=(k == KT - 1))
            for b in range(B_LOC):
                sms = softmax_quad(ps[b], f"qp{grp}{b}", alt=b)
                for hi in range(4):
                    queries[(b, hi)] = sms[hi]
            return queries

        def emit_kp_half(b, half):
            """kp + softmax -> keyp tiles for kv heads 4*half..4*half+3.
            half 0 reads only KV chunk 0, half 1 only chunk 1, so each can be
            emitted as soon as its chunk exists."""
            kp_ps = psum.tile([128, 512], F32, name=f"kp_{b}_{half}", tag="ps")
            for gi in range(4):
                g = half * 4 + gi
                nc.tensor.matmul(kp_ps[:, gi * 128:(gi + 1) * 128],
                                 kv_slice(b, g * 128),
                                 proj_sb[:],
                                 start=True, stop=True)
            # copy_first=False: skips a 690ns DVE copy per (b, half) in the
            # post-chunk window where DVE queue latency gates the PE.
            return softmax_quad(kp_ps, f"kp{b}{half}", copy_first=False,
                                alt=half, out_tag="keyp", out_bufs=32)

        def emit_sT_pre(b, half, keyp_half):
            """keyp transposes + PSUM->SBUF copies for (b, half). Emitted in
            the tail of the kv chunk producing this half's v heads, so the
            copies complete while the chunk's last k-tiles still run."""
            kT_ps = psum.tile([128, 512], FP16, name=f"kT_{b}_{half}", tag="ps")
            for gi in range(4):
                nc.tensor.transpose(kT_ps[:, gi * 128:(gi + 1) * 128],
                                    keyp_half[gi][:], ident[:])
            kTs = []
            for gi in range(4):
                kt = small.tile([128, 128], FP16, name=f"kT_{b}_{half}_{gi}",
                                tag="kTs", bufs=16)
                if (gi + b) % 2 == 0:
                    nc.scalar.copy(kt[:], kT_ps[:, gi * 128:(gi + 1) * 128])
                else:
                    nc.vector.tensor_copy(kt[:],
                                          kT_ps[:, gi * 128:(gi + 1) * 128])
                kTs.append(kt)
            return kTs

        def emit_sT_post(b, half, kTs):
            """scoresT matmuls for (b, half); needs the kv chunk's drained
            xv columns (lhsT) + the pre-copied kT tiles (rhs)."""
            sT_ps = psum.tile([128, 512], F32, name=f"sT_{b}_{half}", tag="ps")
            for gi in range(4):
                g = half * 4 + gi
                nc.tensor.matmul(sT_ps[:, gi * 128:(gi + 1) * 128],
                                 kv_slice(b, NKV * HD + g * 128),
                                 kTs[gi][:],
                                 start=True, stop=True)
            for gi in range(4):
                g = half * 4 + gi
                col = (b * NKV + g) * 128
                if (gi + b) % 2 == 0:
                    nc.vector.tensor_copy(sT_sb[:, col:col + 128],
                                          sT_ps[:, gi * 128:(gi + 1) * 128])
                else:
                    nc.scalar.copy(sT_sb[:, col:col + 128],
                                   sT_ps[:, gi * 128:(gi + 1) * 128])

        def emit_out2T(grp, queries):
            """out2T for the 4 heads of group grp (kv head g == grp)."""
            g = grp
            for b in range(B_LOC):
                o2_ps = psum.tile([128, 512], F32, name=f"o2_{grp}_{b}", tag="ps")
                scol = (b * NKV + g) * 128
                for hi in range(4):
                    nc.tensor.matmul(o2_ps[:, hi * 128:(hi + 1) * 128],
                                     queries[(b, hi)][:],
                                     sT_sb[:, scol:scol + 128],
                                     start=True, stop=True)
                for hi in range(4):
                    h = grp * 4 + hi
                    dst = attnT_sb[:, h * TOK + b * 128: h * TOK + (b + 1) * 128]
                    if (hi + b) % 2 == 0:
                        nc.vector.tensor_copy(dst, o2_ps[:, hi * 128:(hi + 1) * 128])
                    else:
                        nc.scalar.copy(dst, o2_ps[:, hi * 128:(hi + 1) * 128])

        def emit_oproj_chunk(co, engine_alt, mid_cbs=()):
            """out[:, co*512:(co+1)*512] = attnT^T @ Wo chunk.

            mid_cbs: [(split_at, cb), ...] — cb emitted after a-tiles below
            split_at; the preceding matmuls are PE filler while the late
            attention chains finish."""
            wq = dma_quarters(wo_d, co, "wo")
            ps = [psum.tile([128, 512], F32, name=f"ops_{co}_{b}", tag="ps")
                  for b in range(B_LOC)]
            bounds = [0] + [s for s, _ in mid_cbs] + [KT]
            cbs = [cb for _, cb in mid_cbs] + [None]
            for rng, cb in zip([range(a, b) for a, b in zip(bounds, bounds[1:])],
                               cbs):
                for a in rng:
                    wt = wq[a // KQ][:, a % KQ, :]
                    for b in range(B_LOC):
                        nc.tensor.matmul(ps[b][:],
                                         attnT_sb[:, a * TOK + b * 128: a * TOK + (b + 1) * 128],
                                         wt,
                                         start=(a == 0), stop=(a == KT - 1))
                if cb is not None:
                    cb()
            for b in range(B_LOC):
                ost = small.tile([128, 512], F32, name=f"ost_{co}_{b}", tag="ost",
                                 bufs=5)
                if (b + engine_alt) % 2 == 0:
                    nc.scalar.copy(ost[:], ps[b][:])
                else:
                    nc.vector.tensor_copy(ost[:], ps[b][:])
                nc.gpsimd.dma_start(out=out_d[b * 128:(b + 1) * 128,
                                              co * 512:(co + 1) * 512],
                                    in_=ost[:])

        def emit_oproj_last(co):
            """Final o-proj chunk, b-outer: each batch tile's contraction
            finishes 1/4-chunk apart so its drain + output DMA overlap the
            remaining batches' matmuls. The very last batch fans its drain +
            output DMA across engines/queues in 128-col slices to minimize
            the post-matmul tail."""
            wq = dma_quarters(wo_d, co, "wo")
            for b in range(B_LOC - 1):
                ps = psum.tile([128, 512], F32, name=f"ops_{co}_{b}", tag="ps")
                for a in range(KT):
                    nc.tensor.matmul(ps[:],
                                     attnT_sb[:, a * TOK + b * 128: a * TOK + (b + 1) * 128],
                                     wq[a // KQ][:, a % KQ, :],
                                     start=(a == 0), stop=(a == KT - 1))
                ost = small.tile([128, 512], F32, name=f"ost_{co}_{b}", tag="ost",
                                 bufs=5)
                # Spread the earlier batches' 1MB outputs across three queues
                # so no single queue has a serialized multi-MB flush right
                # before the kernel-end barrier.
                if b % 2 == 0:
                    nc.scalar.copy(ost[:], ps[:])
                else:
                    nc.vector.tensor_copy(ost[:], ps[:])
                [nc.gpsimd, nc.scalar, nc.sync][b].dma_start(
                    out=out_d[b * 128:(b + 1) * 128,
                              co * 512:(co + 1) * 512],
                    in_=ost[:])
            # Final batch: parallel ACT/DVE drains into two SEPARATE tiles
            # (one shared tile serializes them via a pool dependency), then
            # the two 0.5MB halves ride two warm queues concurrently.
            b = B_LOC - 1
            ps = psum.tile([128, 512], F32, name=f"ops_{co}_{b}", tag="ps")
            for a in range(KT):
                nc.tensor.matmul(ps[:],
                                 attnT_sb[:, a * TOK + b * 128: a * TOK + (b + 1) * 128],
                                 wq[a // KQ][:, a % KQ, :],
                                 start=(a == 0), stop=(a == KT - 1))
            for i, (copy_op, qeng) in enumerate(
                    [(lambda d, s: nc.scalar.copy(d, s), nc.sync),
                     (lambda d, s: nc.vector.tensor_copy(d, s), nc.scalar)]):
                ostq = small.tile([128, 256], F32, name=f"ostq_{co}_{i}",
                                  tag="ostq", bufs=2)
                copy_op(ostq[:], ps[:, i * 256:(i + 1) * 256])
                qeng.dma_start(
                    out=out_d[b * 128:(b + 1) * 128,
                              co * 512 + i * 256: co * 512 + (i + 1) * 256],
                    in_=ostq[:])

        # ---- schedule -------------------------------------------------------
        # Attention stages trail their producers by >= one ~28us PE stage so
        # the cross-engine softmax chains stay off the PE critical path.
        kp_half0, kp_half1 = {}, {}
        emit_kv_chunk(0, pre_all=wkv0_q,
                      post_cb=lambda b: kp_half0.__setitem__(
                          b, emit_kp_half(b, 0)))
        emit_kv_chunk(1,
                      post_cb=lambda b: kp_half1.__setitem__(
                          b, emit_kp_half(b, 1)))

        kTs0, kTs1 = {}, {}

        def _sT0_pre():
            for b in range(B_LOC):
                kTs0[b] = emit_sT_pre(b, 0, kp_half0[b])

        def _sT1_pre():
            for b in range(B_LOC):
                kTs1[b] = emit_sT_pre(b, 1, kp_half1[b])

        emit_kv_chunk(2, tail_cb=_sT0_pre,
                      post_cb=lambda b: emit_sT_post(b, 0, kTs0[b]))
        emit_kv_chunk(3, tail_cb=_sT1_pre,
                      post_cb=lambda b: emit_sT_post(b, 1, kTs1[b]))
        queries = {0: emit_q_chunk(0)}
        queries[1] = emit_q_chunk(1)
        emit_out2T(0, queries.pop(0))
        queries[2] = emit_q_chunk(2)
        emit_out2T(1, queries.pop(1))
        queries[3] = emit_q_chunk(3)
        emit_out2T(2, queries.pop(2))
        queries[4] = emit_q_chunk(4)
        emit_out2T(3, queries.pop(3))
        queries[5] = emit_q_chunk(5)
        emit_out2T(4, queries.pop(4))
        queries[6] = emit_q_chunk(6)
        emit_out2T(5, queries.pop(5))
        queries[7] = emit_q_chunk(7)
        emit_out2T(6, queries.pop(6))

        def _tail_g7():
            emit_out2T(7, queries.pop(7))

        emit_oproj_chunk(0, 0, mid_cbs=[(16, _tail_g7)])
        for co in range(1, H // 512 - 1):
            emit_oproj_chunk(co, co % 2)
        emit_oproj_last(H // 512 - 1)


_NC_CACHE = None


def _get_program():
    global _NC_CACHE
    if _NC_CACHE is None:
        _NC_CACHE = _build_program()
    return _NC_CACHE


def _pack_chunks(w):
    """[H, C] row-major -> [C/512, 128, KT*512] chunk-contiguous fp16."""
    C = w.shape[1]
    return np.ascontiguousarray(
        w.reshape(KT, 128, C // 512, 512).transpose(2, 1, 0, 3)
        .reshape(C // 512, 128, KT * 512)).astype(np.float16)


def kernel(hidden_states, k_cache=None, v_cache=None, mask=None, qkv_w=None,
           o_w=None, proj=None, kv_write_indices=None, **_ignored):
    hidden_states = np.asarray(hidden_states, dtype=np.float32)
    qkv_w = np.asarray(qkv_w, dtype=np.float32)
    o_w = np.asarray(o_w, dtype=np.float32)
    proj = np.asarray(proj, dtype=np.float32)

    # Fold the DCT projection into the q-side weights (exact in fp32).
    wq = qkv_w[:, :QC]
    wqp = (wq.reshape(H, NH, HD) @ proj).reshape(H, QC)
    wqp16 = _pack_chunks(wqp)
    wkv16 = _pack_chunks(qkv_w[:, QC:])
    wo16 = _pack_chunks(o_w)
    proj16 = proj.astype(np.float16)

    in_maps = []
    for c in range(N_CORES):
        shard = hidden_states[c * B_LOC:(c + 1) * B_LOC]          # [4,128,4096]
        # pack xT as [p, k, tok]: row h = k*128+p, tok = b*128+t
        xT = np.ascontiguousarray(
            shard.reshape(B_LOC, T, KT, 128).transpose(3, 2, 0, 1)
            .reshape(128, KT * TOK))
        in_maps.append({
            "xT": xT.astype(np.float16),
            "wqp": wqp16,
            "wkv": wkv16,
            "wo": wo16,
            "proj": proj16,
        })

    nc = _get_program()
    res = run_bass_kernel_spmd(nc, in_maps, list(range(N_CORES)))

    out = np.empty((B, T, H), np.float32)
    for c in range(N_CORES):
        out[c * B_LOC:(c + 1) * B_LOC] = res.results[c]["out"].reshape(B_LOC, T, H)
    kernel.last_results = res
    return out


# revision 23
# speedup vs baseline: 1.0076x; 1.0045x over previous
"""Trainium2 Bass kernel for the ExomaAttention (DCT-kernelized attention) module.

Full-input contract: kernel(**inputs) takes the unsharded inputs and returns
the full [32, 128, 4096] float32 output.

Sharding: pure data-parallel over batch. 8 cores x 4 batches each. Each core
runs an identical Bass program; only the activation shard (hidden_states^T)
differs per core. Weights are replicated. No collectives.

Math notes (validated against the reference in numpy):
  * kv_write_indices == arange(128) == S, so the kv caches are fully
    overwritten by the projected k/v; the k_cache/v_cache/mask inputs are dead.
  * q-side DCT projection folds into the weights on the host:
      qp = (X @ Wq_h) @ proj = X @ (Wq_h @ proj)  per head block h,
    so the Q projection directly produces softmax-ready qp in [token, e]
    layout. The k-side cannot fold (proj contracts the token axis there).
  * Per (b, kv-head g):   kp = k^T @ proj; keyp = softmax_rows(kp)
                          scoresT = v^T @ keyp^T        (one PE transpose of keyp)
    Per (b, head h), g=h//4: query = softmax_rows(qp)
                          out2T[j,i] = sum_t query[t,j] * scoresT[t,i]
    attnT[h*128+j, b*128+i] = out2T[j,i];  out = attnT^T @ o_w
  * All matmul operands are fp16 (fp32 PSUM accumulation): 4x faster PE than
    fp32, ~9e-4 end-to-end relative error.

Perf notes (v1):
  * Weights and xT are repacked HOST-SIDE into chunk-contiguous layout so
    each weight DMA moves a [128, 8*512] quarter-chunk with 8 KiB contiguous
    per-partition lines (vs 1 KiB lines when slicing the row-major matrix).
    The v0 kernel's DMA engines were ~98% time-active at ~1 KiB/packet
    (packet-overhead-bound), causing startup PE stalls and HAM re-throttles.
  * kv-chunk PSUM drains alternate ACT/DVE engines and scoresT emission is
    split per v-half so it trails kv chunks 2/3 immediately (kills the
    ~2us PSUM-wait stall + ~10us half-rate HAM window at the q transition).
  * The last o-proj chunk runs b-outer (contraction completes per batch
    tile 4x earlier) so the final PSUM drains + output DMAs overlap the
    remaining batches' matmuls: tail shrinks from ~7.4us to ~3.5us.
"""

import numpy as np

import concourse.bass as bass
import concourse.mybir as mybir
import concourse.tile as tile
from concourse import bacc
from concourse.bass_utils import run_bass_kernel_spmd
from concourse.masks import make_identity

FP16 = mybir.dt.float16
F32 = mybir.dt.float32
AX_X = mybir.AxisListType.X
EXP = mybir.ActivationFunctionType.Exp

N_CORES = 8
B, T, H = 32, 128, 4096
NH, NKV, HD = 32, 8, 128
B_LOC = B // N_CORES          # 4 batches per core
TOK = B_LOC * T               # 512 tokens per core
KT = H // 128                 # 32 contraction tiles
QC = NH * HD                  # 4096 q columns
KVC = 2 * NKV * HD            # 2048 k+v columns
NQ = 4                        # quarters per 512-col chunk (8 k-tiles each)
KQ = KT // NQ                 # k-tiles per quarter


def _build_program():
    nc = bacc.Bacc("TRN2", target_bir_lowering=False, debug=False)
    # xT packed host-side as [p, k, tok] so one DMA piece has 8 KiB lines.
    xT_d = nc.dram_tensor("xT", [128, KT * TOK], FP16, kind="ExternalInput").ap()
    # weights packed host-side as [chunk, p, kt, 512] (chunk-contiguous).
    wqp_d = nc.dram_tensor("wqp", [QC // 512, 128, KT * 512], FP16,
                           kind="ExternalInput").ap()
    wkv_d = nc.dram_tensor("wkv", [KVC // 512, 128, KT * 512], FP16,
                           kind="ExternalInput").ap()
    wo_d = nc.dram_tensor("wo", [H // 512, 128, KT * 512], FP16,
                          kind="ExternalInput").ap()
    proj_d = nc.dram_tensor("proj", [HD, HD], FP16, kind="ExternalInput").ap()
    out_d = nc.dram_tensor("out", [TOK, H], F32, kind="ExternalOutput").ap()

    with tile.TileContext(nc) as tc:
        _emit(tc, nc, xT_d, wqp_d, wkv_d, wo_d, proj_d, out_d)
    nc.compile()
    return nc


def _emit(tc, nc, xT_d, wqp_d, wkv_d, wo_d, proj_d, out_d):
    from contextlib import ExitStack

    ctx = ExitStack()
    with ctx:
        persist = ctx.enter_context(tc.tile_pool(name="persist", bufs=1))
        wstream = ctx.enter_context(tc.tile_pool(name="wstream", bufs=6))
        small = ctx.enter_context(tc.tile_pool(name="small", bufs=8))
        psum = ctx.enter_context(tc.tile_pool(name="psum", bufs=8, space="PSUM"))

        # ---- resident tiles -------------------------------------------------
        xT_sb = persist.tile([128, KT * TOK], FP16, name="xT_sb", tag="xT_sb")
        xT_v = xT_sb.rearrange("p (k n) -> p k n", n=TOK)          # [128, 32, 512]
        xT_src = xT_d.rearrange("p (k n) -> p k n", n=TOK)
        proj_sb = persist.tile([128, HD], FP16, name="proj_sb", tag="proj_sb")
        # Startup DMA choreography. kv chunk 0 consumes (xT slab k, wkv0
        # tile k) in k order at ~300 GB/s — close to the aggregate DMA
        # ceiling — so both streams are cut into 2-slab pairs (2 KiB lines)
        # and fed in consumption order on two parallel queues: xT on sync
        # (whose HW ring starts ~2.7us earlier), wkv0 on scalar. The first
        # two wkv0 pairs ride on sync so the scalar ring's late start
        # doesn't gate k=0.
        nc.sync.dma_start(out=proj_sb[:], in_=proj_d[:])
        wkv0_q = [wstream.tile([128, KQ * 512], FP16, name=f"wkv_0_{q}", tag="w")
                  for q in range(NQ)]

        def _wkv0_dma(eng, k0, k1):
            q, c0 = k0 // KQ, (k0 % KQ) * 512
            eng.dma_start(out=wkv0_q[q][:, c0:c0 + (k1 - k0) * 512],
                          in_=wkv_d[0, :, k0 * 512:k1 * 512])

        # sync (ring live ~2.7us before scalar's): single-slab DMAs for k=0,1
        # of both streams so the very first matmul unblocks ~2us earlier,
        # then the rest of xT; scalar: the rest of wkv chunk 0. 2-slab pairs
        # (2 KiB lines) up to k=8, 4-slab quads beyond.
        nc.sync.dma_start(out=xT_v[:, 0:2, :], in_=xT_src[:, 0:2, :])
        _wkv0_dma(nc.sync, 0, 2)
        for k0, k1 in ((2, 4), (4, 6), (6, 8), (8, 12), (12, 16), (16, 20),
                       (20, 24), (24, 28), (28, 32)):
            nc.sync.dma_start(out=xT_v[:, k0:k1, :], in_=xT_src[:, k0:k1, :])
        for k0, k1 in ((2, 4), (4, 6), (6, 8), (8, 12), (12, 16), (16, 20),
                       (20, 24), (24, 28), (28, 32)):
            _wkv0_dma(nc.scalar, k0, k1)

        ident = persist.tile([128, 128], FP16, name="ident", tag="ident")
        make_identity(nc, ident[:])

        # PE warm-up: dummy matmuls with no DMA dependency keep the PE busy
        # (and the HAM clock-gate warming) while the first input DMAs land.
        warm = persist.tile([128, 512], FP16, name="warm", tag="warm")
        nc.vector.memset(warm[:], 0.0)
        warm_ps = psum.tile([128, 512], F32, name="warm_ps", tag="ps")
        for _ in range(10):
            nc.tensor.matmul(warm_ps[:], ident[:], warm[:], start=True, stop=True)

        KV_sb = persist.tile([128, B_LOC * KVC], FP16, name="KV_sb", tag="KV_sb")
        attnT_sb = persist.tile([128, NH * TOK], FP16, name="attnT_sb", tag="attnT_sb")
        # scoresT per (b, g): [128, 128] at column (b*NKV+g)*128
        sT_sb = persist.tile([128, B_LOC * NKV * 128], FP16, name="sT_sb", tag="sT_sb")

        def kv_slice(b, col, width=128):
            return KV_sb[:, b * KVC + col: b * KVC + col + width]

        # ---- weight streaming ----------------------------------------------
        def dma_quarters(w_d, ci, pfx, pre_all=None):
            """Fetch chunk ci of a packed weight tensor as 4 quarter tiles."""
            bufs = []
            for q in range(NQ):
                if pre_all is not None:
                    bufs.append(pre_all[q].rearrange("p (k n) -> p k n", n=512))
                    continue
                wt = wstream.tile([128, KQ * 512], FP16,
                                  name=f"{pfx}_{ci}_{q}", tag="w")
                nc.sync.dma_start(out=wt[:],
                                  in_=w_d[ci, :, q * KQ * 512:(q + 1) * KQ * 512])
                bufs.append(wt.rearrange("p (k n) -> p k n", n=512))
            return bufs

        def drain_halves(dst, ps_tile, b):
            """PSUM->SBUF drain split across ACT+DVE so per-batch latency is
            halved and downstream PE consumers unblock sooner."""
            if b % 2 == 0:
                nc.scalar.copy(dst[:, :256], ps_tile[:, :256])
                nc.vector.tensor_copy(dst[:, 256:], ps_tile[:, 256:])
            else:
                nc.vector.tensor_copy(dst[:, :256], ps_tile[:, :256])
                nc.scalar.copy(dst[:, 256:], ps_tile[:, 256:])

        # ---- stage emitters -------------------------------------------------
        def emit_kv_chunk(ci, pre_all=None, tail_cb=None, post_cb=None):
            """KV[:, ci*512:(ci+1)*512] = X @ Wkv chunk for all local batches.

            tail_cb: emitted after k-tile KT-3 — PE work there (e.g. keyp
            transposes) overlaps the chunk's last k-tiles so its cross-engine
            copies complete before the chunk's PSUMs drain.
            post_cb(b): emitted right after batch b's drain, so per-batch
            consumers (kp / scoresT matmuls) start as soon as THEIR columns
            exist instead of after all four drains."""
            wq = dma_quarters(wkv_d, ci, "wkv", pre_all=pre_all)
            ps = [psum.tile([128, 512], F32, name=f"kvps_{ci}_{b}", tag="ps")
                  for b in range(B_LOC)]
            for k in range(KT):
                wt = wq[k // KQ][:, k % KQ, :]
                for b in range(B_LOC):
                    nc.tensor.matmul(ps[b][:],
                                     xT_v[:, k, b * 128:(b + 1) * 128],
                                     wt,
                                     start=(k == 0), stop=(k == KT - 1))
                if tail_cb is not None and k == KT - 6:
                    tail_cb()
            # All drains first: the engine queues are FIFO, so any post work
            # (kp/sT chains with heavy ACT/DVE ops) emitted between drains
            # would delay the later batches' drains and stall the PE.
            for b in range(B_LOC):
                drain_halves(kv_slice(b, ci * 512, 512), ps[b], b)
            if post_cb is not None:
                for b in range(B_LOC):
                    post_cb(b)

        def softmax_quad(ps_tile, pfx, copy_first=True, alt=0,
                         out_tag="soft", out_bufs=48):
            """Row-softmax of 4 [128,128] slices of a [128,512] PSUM tile.
            copy_first: one DVE copy frees the PSUM bank early; the chain then
            runs off the SBUF copy (use where PSUM slot reuse gates the PE)."""
            if copy_first:
                sb = small.tile([128, 512], F32, name=f"{pfx}_sb", tag="smsb",
                                bufs=8)
                nc.vector.tensor_copy(sb[:], ps_tile[:])
                ps_tile = sb
            negmax = small.tile([128, 4], F32, name=f"{pfx}_nm", tag="negmax")
            nc.vector.reduce_max(negmax[:],
                                 ps_tile.rearrange("p (h e) -> p h e", e=128),
                                 axis=AX_X, negate=True)
            exb = small.tile([128, 512], F32, name=f"{pfx}_exb", tag="exp",
                             bufs=8)
            exps = []
            for i in range(4):
                ex = exb[:, i * 128:(i + 1) * 128]
                nc.scalar.activation(ex, ps_tile[:, i * 128:(i + 1) * 128],
                                     EXP, bias=negmax[:, i:i + 1])
                exps.append(ex)
            sums = small.tile([128, 4], F32, name=f"{pfx}_sum", tag="sums")
            nc.vector.reduce_sum(sums[:],
                                 exb.rearrange("p (h e) -> p h e", e=128),
                                 axis=AX_X)
            recip = small.tile([128, 4], F32, name=f"{pfx}_rcp", tag="recip")
            nc.vector.reciprocal(recip[:], sums[:])
            outs = []
            for i in range(4):
                sm = small.tile([128, 128], FP16, name=f"{pfx}_sm{i}",
                                tag=out_tag, bufs=out_bufs)
                if (i + alt) % 2 == 0:
                    nc.vector.tensor_scalar_mul(sm[:], exps[i][:],
                                                recip[:, i:i + 1])
                else:
                    nc.scalar.mul(sm[:], exps[i][:], recip[:, i:i + 1])
                outs.append(sm)
            return outs

        def emit_q_chunk(grp):
            """qp for heads 4*grp..4*grp+3, all batches, + softmax -> query tiles.

            qp[t, e] = X @ Wq' directly (proj folded into Wq on the host), in
            [token, e] layout, which is exactly the out2T lhsT layout.
            """
            queries = {}
            wq = dma_quarters(wqp_d, grp, "wqp")
            ps = [psum.tile([128, 512], F32, name=f"qps_{grp}_{b}", tag="ps")
                  for b in range(B_LOC)]
            for k in range(KT):
                wt = wq[k // KQ][:, k % KQ, :]
                for b in range(B_LOC):
                    nc.tensor.matmul(ps[b][:],
                                     xT_v[:, k, b * 128:(b + 1) * 128],
                                     wt,
                                     start=(k == 0), stop=(k == KT - 1))
            for b in range(B_LOC):
                sms = softmax_quad(ps[b], f"qp{grp}{b}", alt=b)
                for hi in range(4):
                    queries[(b, hi)] = sms[hi]
            return queries

        def emit_kp_half(b, half):
            """kp + softmax -> keyp tiles for kv heads 4*half..4*half+3.
            half 0 reads only KV chunk 0, half 1 only chunk 1, so each can be
            emitted as soon as its chunk exists."""
            kp_ps = psum.tile([128, 512], F32, name=f"kp_{b}_{half}", tag="ps")
            for gi in range(4):
                g = half * 4 + gi
                nc.tensor.matmul(kp_ps[:, gi * 128:(gi + 1) * 128],
                                 kv_slice(b, g * 128),
                                 proj_sb[:],
                                 start=True, stop=True)
            # copy_first=False: skips a 690ns DVE copy per (b, half) in the
            # post-chunk window where DVE queue latency gates the PE.
            return softmax_quad(kp_ps, f"kp{b}{half}", copy_first=False,
                                alt=half, out_tag="keyp", out_bufs=32)

        def emit_sT_pre(b, half, keyp_half):
            """keyp transposes + PSUM->SBUF copies for (b, half). Emitted in
            the tail of the kv chunk producing this half's v heads, so the
            copies complete while the chunk's last k-tiles still run."""
            kT_ps = psum.tile([128, 512], FP16, name=f"kT_{b}_{half}", tag="ps")
            for gi in range(4):
                nc.tensor.transpose(kT_ps[:, gi * 128:(gi + 1) * 128],
                                    keyp_half[gi][:], ident[:])
            kTs = []
            for gi in range(4):
                kt = small.tile([128, 128], FP16, name=f"kT_{b}_{half}_{gi}",
                                tag="kTs", bufs=16)
                if (gi + b) % 2 == 0:
                    nc.scalar.copy(kt[:], kT_ps[:, gi * 128:(gi + 1) * 128])
                else:
                    nc.vector.tensor_copy(kt[:],
                                          kT_ps[:, gi * 128:(gi + 1) * 128])
                kTs.append(kt)
            return kTs

        def emit_sT_post(b, half, kTs):
            """scoresT matmuls for (b, half); needs the kv chunk's drained
            xv columns (lhsT) + the pre-copied kT tiles (rhs)."""
            sT_ps = psum.tile([128, 512], F32, name=f"sT_{b}_{half}", tag="ps")
            for gi in range(4):
                g = half * 4 + gi
                nc.tensor.matmul(sT_ps[:, gi * 128:(gi + 1) * 128],
                                 kv_slice(b, NKV * HD + g * 128),
                                 kTs[gi][:],
                                 start=True, stop=True)
            for gi in range(4):
                g = half * 4 + gi
                col = (b * NKV + g) * 128
                if (gi + b) % 2 == 0:
                    nc.vector.tensor_copy(sT_sb[:, col:col + 128],
                                          sT_ps[:, gi * 128:(gi + 1) * 128])
                else:
                    nc.scalar.copy(sT_sb[:, col:col + 128],
                                   sT_ps[:, gi * 128:(gi + 1) * 128])

        def emit_out2T(grp, queries):
            """out2T for the 4 heads of group grp (kv head g == grp)."""
            g = grp
            for b in range(B_LOC):
                o2_ps = psum.tile([128, 512], F32, name=f"o2_{grp}_{b}", tag="ps")
                scol = (b * NKV + g) * 128
                for hi in range(4):
                    nc.tensor.matmul(o2_ps[:, hi * 128:(hi + 1) * 128],
                                     queries[(b, hi)][:],
                                     sT_sb[:, scol:scol + 128],
                                     start=True, stop=True)
                for hi in range(4):
                    h = grp * 4 + hi
                    dst = attnT_sb[:, h * TOK + b * 128: h * TOK + (b + 1) * 128]
                    if (hi + b) % 2 == 0:
                        nc.vector.tensor_copy(dst, o2_ps[:, hi * 128:(hi + 1) * 128])
                    else:
                        nc.scalar.copy(dst, o2_ps[:, hi * 128:(hi + 1) * 128])

        def emit_oproj_chunk(co, engine_alt, mid_cbs=()):
            """out[:, co*512:(co+1)*512] = attnT^T @ Wo chunk.

            mid_cbs: [(split_at, cb), ...] — cb emitted after a-tiles below
            split_at; the preceding matmuls are PE filler while the late
            attention chains finish."""
            wq = dma_quarters(wo_d, co, "wo")
            ps = [psum.tile([128, 512], F32, name=f"ops_{co}_{b}", tag="ps")
                  for b in range(B_LOC)]
            bounds = [0] + [s for s, _ in mid_cbs] + [KT]
            cbs = [cb for _, cb in mid_cbs] + [None]
            for rng, cb in zip([range(a, b) for a, b in zip(bounds, bounds[1:])],
                               cbs):
                for a in rng:
                    wt = wq[a // KQ][:, a % KQ, :]
                    for b in range(B_LOC):
                        nc.tensor.matmul(ps[b][:],
                                         attnT_sb[:, a * TOK + b * 128: a * TOK + (b + 1) * 128],
                                         wt,
                                         start=(a == 0), stop=(a == KT - 1))
                if cb is not None:
                    cb()
            for b in range(B_LOC):
                ost = small.tile([128, 512], F32, name=f"ost_{co}_{b}", tag="ost",
                                 bufs=5)
                if (b + engine_alt) % 2 == 0:
                    nc.scalar.copy(ost[:], ps[b][:])
                else:
                    nc.vector.tensor_copy(ost[:], ps[b][:])
                nc.gpsimd.dma_start(out=out_d[b * 128:(b + 1) * 128,
                                              co * 512:(co + 1) * 512],
                                    in_=ost[:])

        def emit_oproj_last(co):
            """Final o-proj chunk, b-outer: each batch tile's contraction
            finishes 1/4-chunk apart so its drain + output DMA overlap the
            remaining batches' matmuls. The very last batch fans its drain +
            output DMA across engines/queues in 128-col slices to minimize
            the post-matmul tail."""
            wq = dma_quarters(wo_d, co, "wo")
            for b in range(B_LOC - 1):
                ps = psum.tile([128, 512], F32, name=f"ops_{co}_{b}", tag="ps")
                for a in range(KT):
                    nc.tensor.matmul(ps[:],
                                     attnT_sb[:, a * TOK + b * 128: a * TOK + (b + 1) * 128],
                                     wq[a // KQ][:, a % KQ, :],
                                     start=(a == 0), stop=(a == KT - 1))
                ost = small.tile([128, 512], F32, name=f"ost_{co}_{b}", tag="ost",
                                 bufs=5)
                # Spread the earlier batches' 1MB outputs across three queues
                # so no single queue has a serialized multi-MB flush right
                # before the kernel-end barrier.
                if b % 2 == 0:
                    nc.scalar.copy(ost[:], ps[:])
                else:
                    nc.vector.tensor_copy(ost[:], ps[:])
                [nc.gpsimd, nc.scalar, nc.sync][b].dma_start(
                    out=out_d[b * 128:(b + 1) * 128,
                              co * 512:(co + 1) * 512],
                    in_=ost[:])
            # Final batch: parallel ACT/DVE drains into two SEPARATE tiles
            # (one shared tile serializes them via a pool dependency), then
            # the two 0.5MB halves ride two warm queues concurrently.
            b = B_LOC - 1
            ps = psum.tile([128, 512], F32, name=f"ops_{co}_{b}", tag="ps")
            for a in range(KT):
                nc.tensor.matmul(ps[:],
                                 attnT_sb[:, a * TOK + b * 128: a * TOK + (b + 1) * 128],
                                 wq[a // KQ][:, a % KQ, :],
                                 start=(a == 0), stop=(a == KT - 1))
            for i, (copy_op, qeng) in enumerate(
                    [(lambda d, s: nc.scalar.copy(d, s), nc.sync),
                     (lambda d, s: nc.vector.tensor_copy(d, s), nc.scalar)]):
                ostq = small.tile([128, 256], F32, name=f"ostq_{co}_{i}",
                                  tag="ostq", bufs=2)
                copy_op(ostq[:], ps[:, i * 256:(i + 1) * 256])
                qeng.dma_start(
                    out=out_d[b * 128:(b + 1) * 128,
                              co * 512 + i * 256: co * 512 + (i + 1) * 256],
                    in_=ostq[:])

        # ---- schedule -------------------------------------------------------
        # Attention stages trail their producers by >= one ~28us PE stage so
        # the cross-engine softmax chains stay off the PE critical path.
        kp_half0, kp_half1 = {}, {}
        emit_kv_chunk(0, pre_all=wkv0_q,
                      post_cb=lambda b: kp_half0.__setitem__(
                          b, emit_kp_half(b, 0)))
        emit_kv_chunk(1,
                      post_cb=lambda b: kp_half1.__setitem__(
                          b, emit_kp_half(b, 1)))

        kTs0, kTs1 = {}, {}

        def _sT0_pre():
            for b in range(B_LOC):
                kTs0[b] = emit_sT_pre(b, 0, kp_half0[b])

        def _sT1_pre():
            for b in range(B_LOC):
                kTs1[b] = emit_sT_pre(b, 1, kp_half1[b])

        emit_kv_chunk(2, tail_cb=_sT0_pre,
                      post_cb=lambda b: emit_sT_post(b, 0, kTs0[b]))
        emit_kv_chunk(3, tail_cb=_sT1_pre,
                      post_cb=lambda b: emit_sT_post(b, 1, kTs1[b]))
        queries = {0: emit_q_chunk(0)}
        queries[1] = emit_q_chunk(1)
        emit_out2T(0, queries.pop(0))
        queries[2] = emit_q_chunk(2)
        emit_out2T(1, queries.pop(1))
        queries[3] = emit_q_chunk(3)
        emit_out2T(2, queries.pop(2))
        queries[4] = emit_q_chunk(4)
        emit_out2T(3, queries.pop(3))
        queries[5] = emit_q_chunk(5)
        emit_out2T(4, queries.pop(4))
        queries[6] = emit_q_chunk(6)
        emit_out2T(5, queries.pop(5))
        queries[7] = emit_q_chunk(7)
        emit_out2T(6, queries.pop(6))

        def _tail_g7():
            emit_out2T(7, queries.pop(7))

        emit_oproj_chunk(0, 0, mid_cbs=[(16, _tail_g7)])
        for co in range(1, H // 512 - 1):
            emit_oproj_chunk(co, co % 2)
        emit_oproj_last(H // 512 - 1)


_NC_CACHE = None


def _get_program():
    global _NC_CACHE
    if _NC_CACHE is None:
        _NC_CACHE = _build_program()
    return _NC_CACHE


def _pack_chunks(w):
    """[H, C] row-major -> [C/512, 128, KT*512] chunk-contiguous fp16."""
    C = w.shape[1]
    return np.ascontiguousarray(
        w.reshape(KT, 128, C // 512, 512).transpose(2, 1, 0, 3)
        .reshape(C // 512, 128, KT * 512)).astype(np.float16)


def kernel(hidden_states, k_cache=None, v_cache=None, mask=None, qkv_w=None,
           o_w=None, proj=None, kv_write_indices=None, **_ignored):
    hidden_states = np.asarray(hidden_states, dtype=np.float32)
    qkv_w = np.asarray(qkv_w, dtype=np.float32)
    o_w = np.asarray(o_w, dtype=np.float32)
    proj = np.asarray(proj, dtype=np.float32)

    # Fold the DCT projection into the q-side weights (exact in fp32).
    wq = qkv_w[:, :QC]
    wqp = (wq.reshape(H, NH, HD) @ proj).reshape(H, QC)
    wqp16 = _pack_chunks(wqp)
    wkv16 = _pack_chunks(qkv_w[:, QC:])
    wo16 = _pack_chunks(o_w)
    proj16 = proj.astype(np.float16)

    in_maps = []
    for c in range(N_CORES):
        shard = hidden_states[c * B_LOC:(c + 1) * B_LOC]          # [4,128,4096]
        # pack xT as [p, k, tok]: row h = k*128+p, tok = b*128+t
        xT = np.ascontiguousarray(
            shard.reshape(B_LOC, T, KT, 128).transpose(3, 2, 0, 1)
            .reshape(128, KT * TOK))
        in_maps.append({
            "xT": xT.astype(np.float16),
            "wqp": wqp16,
            "wkv": wkv16,
            "wo": wo16,
            "proj": proj16,
        })

    nc = _get_program()
    res = run_bass_kernel_spmd(nc, in_maps, list(range(N_CORES)))

    out = np.empty((B, T, H), np.float32)
    for c in range(N_CORES):
        out[c * B_LOC:(c + 1) * B_LOC] = res.results[c]["out"].reshape(B_LOC, T, H)
    kernel.last_results = res
    return out
